# revision 1
# baseline (speedup 1.0000x reference)
"""MoE routing (gate) kernel for Trainium2, 8 NeuronCores, data-parallel.

Computes, for x [65536, 4096] f32 and W [64, 4096] f32:
    logits  = x @ W.T                       # [65536, 64]
    scores  = softmax(logits, axis=-1)
    weights, indices = top_k(scores, 8)     # [65536, 8] each
    weights *= 2.5

Sharding: token dim split 8 ways (8192 tokens/core); W replicated.

Host-side prep packs each core's x shard into the exact SBUF tile
layout so every group load is ONE fully contiguous 8 MiB DMA:
    xb[g, p, k, t] = x[g*T_G + t, k*128 + p]   (per core shard)
and W into wt[p, k, e] = W[e, k*128 + p].

Per-core program (Tile framework), for each group of 512 tokens:
  - one 8 MiB contiguous dma_start -> xg [128, KC, T_G]
  - KC accumulating PE matmuls: logitsT[64, T_G] += wt_k.T @ xg_k
  - copy PSUM->SBUF, 4 PE transposes -> logits [128 tok, 64 exp]
  - DVE max/max_index -> top-8 values + indices (desc order, first-index
    tie-break = jax.lax.top_k order)
  - ACT exp(x - max) with accumulated row-sum -> softmax denominator
  - weights = exp(top8 - max) * 2.5 / denom

Matmul dtype modes (GATE_MODE):
  f32r   - fp32 data streamed as float32r: 1 cyc/row (4x faster than f32)
  split3 - x,W pre-split host-side into fp16 hi+lo planes; logits =
           x1@w1 + x1@w2 + x2@w1 (3 bf16-rate passes, near-fp32 exact)
  f32    - plain fp32 (4 cyc/row), exact
"""

import os
import sys

for _p in ("/opt/trn_rl_repo", "/root/.axon_site/_ro/trn_rl_repo"):
    if os.path.isdir(_p) and _p not in sys.path:
        sys.path.append(_p)

import numpy as np

import concourse.bass as bass
import concourse.mybir as mybir
from concourse import library_config, masks, tile
from concourse.bass_utils import run_bass_kernel_spmd
from concourse.vector_clock import ScopedClock

TOKENS = 65536
D = 4096
E = 64
TOPK = 8
ROUTE_SCALE = 2.5
N_CORES = 8
T_CORE = TOKENS // N_CORES  # 8192
T_G = 512                   # tokens per group (one PSUM bank at fp32)
N_G = T_CORE // T_G         # 16
KC = D // 128               # 32 contraction chunks

F32 = mybir.dt.float32
F32R = mybir.dt.float32r
F16 = mybir.dt.float16
I32 = mybir.dt.int32
U32 = mybir.dt.uint32

MODE = os.environ.get("GATE_MODE", "split3")  # split3 | rescue | f32r | f32

# --- rescue-mode constants -------------------------------------------------
T_G1 = 512                  # tokens per approx group (one PSUM bank at fp32)
N_G1 = T_CORE // T_G1       # 16
THRESH = 3.0e-3             # min-gap flag threshold (~4 sigma of fp16 error)
R_PP = 16                   # rescue slots per partition (capacity 2048)
R_CHUNK = 512               # rescued tokens per gather/matmul chunk
N_RCH = 128 * R_PP // R_CHUNK  # 4 chunks
S_PER_CH = R_CHUNK // 128   # 4 slots per chunk

# ---------------------------------------------------------------------------
# Walrus in this container rejects >1 sync-wait on control instructions; the
# stock TileContext tail drain carries one wait per live processor.  Spread
# them across sync-engine NOPs (1 each) before the drain.
_MAX_WAITS = 1


def _patched_drain_and_barrier(self, tick_clock, wait_clock):
    nc = self.nc
    probe = nc.sync.nop()
    wait_clock.add_sem_waits(probe.ins, ScopedClock({None: tick_clock.global_clock}))
    waits = list(probe.ins.sync_info.on_wait or [])
    probe.ins.sync_info.on_wait = waits[:_MAX_WAITS]
    for i in range(_MAX_WAITS, len(waits), _MAX_WAITS):
        extra = nc.sync.nop()
        if extra.ins.sync_info is None:
            extra.ins.sync_info = mybir.SyncInfo(
                on_wait=waits[i : i + _MAX_WAITS], on_update=[]
            )
        else:
            extra.ins.sync_info.on_wait = waits[i : i + _MAX_WAITS]
    nc.sync.drain()

    nc.all_engine_barrier()
    assert self.sems is not None
    popped = nc._tile_sem_poison_stack.pop()
    assert popped is self._sem_poison
    nc.clear_and_free_semaphores(list(self.sems.allocated().values()))
    nc.all_engine_barrier()


tile.TileContext._drain_and_barrier = _patched_drain_and_barrier


def _split_multi_waits(nc: bass.Bass, max_waits: int = _MAX_WAITS):
    """Walrus here caps sync waits at 1 per instruction (any engine struct).
    Hoist excess waits onto same-engine NOPs inserted just before the
    offending instruction — the sequencer satisfies them in order, so the
    semantics (AND of all waits before execute) are preserved."""
    n = 0
    for fn in nc.m.functions:
        for bb in fn.blocks:
            out = []
            changed = False
            for inst in bb.instructions:
                si = inst.sync_info
                w = list(si.on_wait) if (si and si.on_wait) else []
                if len(w) > max_waits:
                    extras = w[: len(w) - max_waits]
                    si.on_wait = w[len(w) - max_waits :]
                    for i0 in range(0, len(extras), max_waits):
                        nop = mybir.InstNoOp(
                            name=f"I-wsplit-{nc.next_id()}", ins=[], outs=[]
                        )
                        nop.engine = inst.engine
                        nop.sync_info = mybir.SyncInfo(
                            on_wait=extras[i0 : i0 + max_waits], on_update=[]
                        )
                        out.append(nop)
                        n += 1
                    changed = True
                out.append(inst)
            if changed:
                bb.instructions = out
    return n
# ---------------------------------------------------------------------------


def _build_rescue_program() -> bass.Bass:
    """fp16 approx pass + exact rescue of near-tie tokens.

    Phase A: logits ~= x1 @ w1 (hi fp16 planes), full top-8 epilogue for all
      tokens, plus a per-token flag = (min adjacent gap of ranks 1..9 < t).
    Phase B: per-partition compaction of flags into <=R_PP rescue slots;
      token id of slot (p, s) = slot_value*128 + p.
    Phase C: dma_gather(transpose) pulls the flagged tokens' full-precision
      rows (hi|lo fp16), recomputes exact logits (3-term split), redoes the
      epilogue, and indirect-scatters weights/indices over the approx rows.
    """
    nc = bass.Bass()
    xb = nc.declare_dram_parameter("xb", [N_G1, 128, KC, T_G1], F16, isOutput=False)
    xrows = nc.declare_dram_parameter("xrows", [T_CORE, 2 * D], F16, isOutput=False)
    wt = nc.declare_dram_parameter("wt", [128, KC, 2, E], F16, isOutput=False)
    w_out = nc.declare_dram_parameter("w_out", [T_CORE, TOPK], F32, isOutput=True)
    i_out = nc.declare_dram_parameter("i_out", [T_CORE, TOPK], I32, isOutput=True)

    I16 = mybir.dt.int16
    ALU = mybir.AluOpType

    with tile.TileContext(nc) as tc:
        st_sem = nc.alloc_semaphore(name="st_done")
        idx_sem = nc.alloc_semaphore(name="idx_done")
        g_sem = nc.alloc_semaphore(name="g_done")
        sc_sem = nc.alloc_semaphore(name="sc_done")
        lib0 = nc.alloc_semaphore(name="lib_ld0")
        lib1 = nc.alloc_semaphore(name="lib_ld1")
        nc._gate_lib_sems = [lib0, lib1]
        _my_sems = [st_sem, idx_sem, g_sem, sc_sem, lib0, lib1]
        with (
            tc.tile_pool(name="const", bufs=1) as const_pool,
            tc.tile_pool(name="state", bufs=1) as state_pool,
            tc.tile_pool(name="lsb", bufs=2) as lspool,
            tc.tile_pool(name="lg", bufs=4) as lgpool,
            tc.tile_pool(name="epi", bufs=6) as epool,
            tc.tile_pool(name="outg", bufs=2) as opool,
            tc.tile_pool(name="ps_t", bufs=2, space="PSUM") as ps_t,
            tc.tile_pool(name="ps_i", bufs=1, space="PSUM") as ps_i,
        ):
            ident = const_pool.tile([128, 128], F32)
            masks.make_identity(nc, ident[:])
            wt_sb = const_pool.tile([128, KC, 2, E], F16)
            nc.sync.dma_start(wt_sb[:], wt[:])
            # pm128[p] = p - 128 (for slot-value -> token-id arithmetic)
            pm128_i = const_pool.tile([128, 1], I32)
            nc.gpsimd.iota(pm128_i[:], pattern=[[0, 1]], base=-128,
                           channel_multiplier=1)
            pm128 = const_pool.tile([128, 1], F32)
            nc.vector.tensor_copy(pm128[:], pm128_i[:])

            fscore = state_pool.tile([128, 64], F32)
            ids_f = state_pool.tile([128, R_PP], F32)       # token ids (pad<0)
            ids_scat = state_pool.tile([128, R_PP], I32)    # pad -> OOB large
            idx_chunks = [
                state_pool.tile([128, R_CHUNK // 16], I16, name=f"idxc{c}")
                for c in range(N_RCH)
            ]

            def epilogue_tile(ls, j, w_grp, i_grp, flag_slot=None):
                """One 128-token tile: transpose, top-8, softmax; optionally
                write the near-tie flag score into fscore[:, flag_slot]."""
                lt_ps = ps_t.tile([128, E], F32, name="lt_ps")
                nc.tensor.transpose(
                    lt_ps[:], ls[:, j * 128 : (j + 1) * 128], ident[:E, :E]
                )
                lg = lgpool.tile([128, E], F32, tag="lg")
                nc.vector.tensor_copy(lg[:], lt_ps[:])

                mx8 = epool.tile([128, TOPK], F32, tag="mx8")
                nc.vector.max(mx8[:], lg[:])
                nc.vector.max_index(i_grp[:, j, :].bitcast(U32), mx8[:], lg[:])

                if flag_slot is not None:
                    lgrm = lgpool.tile([128, E], F32, tag="lgrm")
                    nc.vector.match_replace(lgrm[:], mx8[:], lg[:], -1e30)
                    mx9 = epool.tile([128, TOPK], F32, tag="mx9")
                    nc.vector.max(mx9[:], lgrm[:])
                    t9 = epool.tile([128, 9], F32, tag="t9")
                    nc.vector.tensor_copy(t9[:, 0:8], mx8[:])
                    nc.vector.tensor_copy(t9[:, 8:9], mx9[:, 0:1])
                    # neg-gaps: rank_{k+1} - rank_k; max8 head = -min(gap)
                    nggaps = epool.tile([128, 8], F32, tag="nggaps")
                    nc.vector.tensor_tensor(
                        out=nggaps[:], in0=t9[:, 1:9], in1=t9[:, 0:8],
                        op=ALU.subtract,
                    )
                    mxg = epool.tile([128, 8], F32, tag="mxg")
                    nc.vector.max(mxg[:], nggaps[:])
                    # fscore[:, s] = (min gap < t) * (s + 1)
                    nc.vector.tensor_scalar(
                        out=fscore[:, flag_slot : flag_slot + 1],
                        in0=mxg[:, 0:1],
                        scalar1=-THRESH,
                        scalar2=float(flag_slot + 1),
                        op0=ALU.is_gt,
                        op1=ALU.mult,
                    )

                negmax = epool.tile([128, 1], F32, tag="negmax")
                nc.scalar.mul(negmax[:], mx8[:, 0:1], -1.0)
                expall = epool.tile([128, E], F32, tag="expall")
                denom = epool.tile([128, 1], F32, tag="denom")
                nc.scalar.activation(
                    expall[:],
                    lg[:],
                    mybir.ActivationFunctionType.Exp,
                    bias=negmax[:],
                    accum_out=denom[:],
                )
                exp8 = epool.tile([128, TOPK], F32, tag="exp8")
                nc.scalar.activation(
                    exp8[:],
                    mx8[:],
                    mybir.ActivationFunctionType.Exp,
                    bias=negmax[:],
                )
                r25 = epool.tile([128, 1], F32, tag="r25")
                nc.vector.reciprocal(r25[:], denom[:])
                nc.scalar.mul(r25[:], r25[:], ROUTE_SCALE)
                nc.vector.tensor_scalar_mul(w_grp[:, j, :], exp8[:], r25[:])

            # ---------------- Phase A: approx pass -------------------------
            with (
                tc.tile_pool(name="xin", bufs=2) as xpool,
                tc.tile_pool(name="ps_l", bufs=2, space="PSUM") as ps_l,
            ):
                for g in range(N_G1):
                    xg = xpool.tile([128, KC, T_G1], F16, tag="xg")
                    nc.sync.dma_start(xg[:], xb[g])
                    logitsT = ps_l.tile([E, T_G1], F32, name="logitsT")
                    for k in range(KC):
                        nc.tensor.matmul(
                            logitsT[:],
                            wt_sb[:, k, 0, :],
                            xg[:, k, :],
                            start=(k == 0),
                            stop=(k == KC - 1),
                        )
                    ls = lspool.tile([E, T_G1], F32, tag="ls")
                    nc.scalar.copy(ls[:], logitsT[:])

                    w_grp = opool.tile([128, T_G1 // 128, TOPK], F32, tag="wg")
                    i_grp = opool.tile([128, T_G1 // 128, TOPK], I32, tag="ig")
                    for j in range(T_G1 // 128):
                        epilogue_tile(
                            ls, j, w_grp, i_grp,
                            flag_slot=g * (T_G1 // 128) + j,
                        )

                    nc.scalar.dma_start(
                        w_out[g * T_G1 : (g + 1) * T_G1, :].rearrange(
                            "(j p) e -> p j e", p=128
                        ),
                        w_grp[:],
                    ).then_inc(st_sem, 16)
                    nc.scalar.dma_start(
                        i_out[g * T_G1 : (g + 1) * T_G1, :].rearrange(
                            "(j p) e -> p j e", p=128
                        ),
                        i_grp[:],
                    ).then_inc(st_sem, 16)

            # ---------------- Phase B: compaction --------------------------
            m0 = state_pool.tile([128, 8], F32)
            nc.vector.max(m0[:], fscore[:])
            f1 = state_pool.tile([128, 64], F32)
            nc.vector.match_replace(f1[:], m0[:], fscore[:], 0.0)
            m1 = state_pool.tile([128, 8], F32)
            nc.vector.max(m1[:], f1[:])
            vals = state_pool.tile([128, R_PP], F32)
            nc.vector.tensor_copy(vals[:, 0:8], m0[:])
            nc.vector.tensor_copy(vals[:, 8:16], m1[:])

            # ids = vals*128 + (p-128); pad (vals==0) -> p-128 < 0
            nc.vector.tensor_scalar(
                out=ids_f[:], in0=vals[:], scalar1=128.0, scalar2=pm128[:, 0:1],
                op0=ALU.mult, op1=ALU.add,
            )
            # gather ids: pad -> 0 (valid dummy row; result discarded)
            idg_f = state_pool.tile([128, R_PP], F32)
            isv = state_pool.tile([128, R_PP], F32)
            nc.vector.tensor_scalar(
                out=isv[:], in0=vals[:], scalar1=0.0, scalar2=None,
                op0=ALU.is_gt,
            )
            nc.vector.tensor_tensor(
                out=idg_f[:], in0=ids_f[:], in1=isv[:], op=ALU.mult
            )
            # scatter ids: pad -> +20000+ (past bounds_check -> skipped)
            pad_of = state_pool.tile([128, R_PP], F32)
            nc.vector.tensor_scalar(
                out=pad_of[:], in0=vals[:], scalar1=0.0, scalar2=20128.0,
                op0=ALU.is_le, op1=ALU.mult,
            )
            ids_scat_f = state_pool.tile([128, R_PP], F32)
            nc.vector.tensor_tensor(
                out=ids_scat_f[:], in0=ids_f[:], in1=pad_of[:], op=ALU.add
            )
            nc.vector.tensor_copy(ids_scat[:], ids_scat_f[:])

            # transpose gather ids to [16, 128] wrap via PE
            idsT_ps = ps_i.tile([R_PP, 128], F32, name="idsT")
            nc.tensor.transpose(idsT_ps[:], idg_f[:], ident[:, :])
            idsT_f = state_pool.tile([R_PP, 128], F32)
            nc.vector.tensor_copy(idsT_f[:], idsT_ps[:])
            idsT_i16 = state_pool.tile([R_PP, 128], I16)
            nc.vector.tensor_copy(idsT_i16[:], idsT_f[:])

            # build per-chunk replicated idx tiles [128, R_CHUNK//16]:
            # chunk slot i = s_local*128 + p lives at [i % 16, i // 16]
            # (interleaved wrap, per bass_interp InstDMAGatherAnt).
            # SBUF APs cannot turn a free-dim stride into a partition step, so
            # bounce through a DRAM scratch where arbitrary APs are legal.
            ids_dram = nc.dram_tensor(
                "ids_scratch", [R_PP, 128], I16, kind="Internal"
            )
            nc.sync.dma_start(ids_dram[:], idsT_i16[:]).then_inc(idx_sem, 16)
            nc.sync.wait_ge(idx_sem, 16)
            for c in range(N_RCH):
                idxc = idx_chunks[c]
                src = ids_dram[c * S_PER_CH : (c + 1) * S_PER_CH, :].rearrange(
                    "s (a q) -> q (s a)", a=8
                )
                nc.sync.dma_start(idxc[0:16, :], src).then_inc(idx_sem, 16)
                nc.sync.dma_start(idxc[16:32, :], idxc[0:16, :]).then_inc(
                    idx_sem, 16
                )
                nc.sync.dma_start(idxc[32:64, :], idxc[0:32, :]).then_inc(
                    idx_sem, 16
                )
                nc.sync.dma_start(idxc[64:128, :], idxc[0:64, :]).then_inc(
                    idx_sem, 16
                )

            # ---------------- Phase C: exact rescue ------------------------
            with (
                tc.tile_pool(name="xres", bufs=2) as gpool,
                tc.tile_pool(name="ps_r", bufs=2, space="PSUM") as ps_r,
            ):
                first_scatter = True
                for c in range(N_RCH):
                    xr = gpool.tile([128, 2 * KC, R_CHUNK], F16, tag="xr")
                    if c == 0:
                        # all 16 idx-build DMAs + the ids_dram store landed
                        nc.gpsimd.wait_ge(idx_sem, 16 * 17)
                    nc.gpsimd.dma_gather(
                        xr[:], xrows[:], idx_chunks[c][:],
                        R_CHUNK, R_CHUNK, 2 * D, transpose=True,
                    ).then_inc(g_sem, 16)
                    # Tile treats the gather as instruction-retire complete;
                    # the matmuls must wait for the DATA to land.
                    nc.tensor.wait_ge(g_sem, 16 * (c + 1))
                    r_ps = ps_r.tile([E, R_CHUNK], F32, name="r_ps")
                    i_mm = 0
                    for k in range(KC):
                        for (xi, wi) in ((0, 0), (0, 1), (1, 0)):
                            nc.tensor.matmul(
                                r_ps[:],
                                wt_sb[:, k, wi, :],
                                xr[:, xi * KC + k, :],
                                start=(i_mm == 0),
                                stop=(i_mm == 3 * KC - 1),
                            )
                            i_mm += 1
                    rls = lspool.tile([E, R_CHUNK], F32, tag="rls")
                    nc.scalar.copy(rls[:], r_ps[:])

                    rw = opool.tile([128, S_PER_CH, TOPK], F32, tag="rw")
                    ri = opool.tile([128, S_PER_CH, TOPK], I32, tag="ri")
                    for j in range(S_PER_CH):
                        epilogue_tile(rls, j, rw, ri, flag_slot=None)

                    if first_scatter:
                        nc.gpsimd.wait_ge(st_sem, 16 * 2 * N_G1)
                        first_scatter = False
                    for j in range(S_PER_CH):
                        s = c * S_PER_CH + j
                        nc.gpsimd.indirect_dma_start(
                            out=w_out[:],
                            out_offset=bass.IndirectOffsetOnAxis(
                                ap=ids_scat[:, s : s + 1], axis=0
                            ),
                            in_=rw[:, j, :],
                            in_offset=None,
                            bounds_check=T_CORE - 1,
                            oob_is_err=False,
                        ).then_inc(sc_sem, 16)
                        nc.gpsimd.indirect_dma_start(
                            out=i_out[:],
                            out_offset=bass.IndirectOffsetOnAxis(
                                ap=ids_scat[:, s : s + 1], axis=0
                            ),
                            in_=ri[:, j, :],
                            in_offset=None,
                            bounds_check=T_CORE - 1,
                            oob_is_err=False,
                        ).then_inc(sc_sem, 16)

                # all rescue scatters must land before the kernel drains
                nc.gpsimd.wait_ge(sc_sem, 16 * 2 * N_RCH * S_PER_CH)
                del _my_sems  # left set at end; runtime zeroes sems per load

    # Raw Bass skips Bacc's compile passes; gpsimd extended insts
    # (dma_gather) need the GPSIMD library loads inserted and the ISA
    # bytes generated, else walrus dies with "ISA wrong length".
    import bass_rust as _bass_rust
    from concourse.library_config import all_libraries, standard

    inst_type_to_lib_mask = {}
    for lib in all_libraries:
        for inst_type in lib.instructions:
            inst_type_to_lib_mask[inst_type] = inst_type_to_lib_mask.get(
                inst_type, 0
            ) | (1 << lib.index)
    _bass_rust.insert_library_loads(
        nc, inst_type_to_lib_mask, len(all_libraries), standard.index
    )
    # The inserted library loads are DMA-classified (ucode fetch) but carry
    # no sem update, which the race detector / BIR invariants reject. Give
    # them a dedicated sem nobody waits on.
    _lib_sems = list(nc._gate_lib_sems)
    for fn in nc.m.functions:
        for bb in fn.blocks:
            for inst in bb.instructions:
                if type(inst).__name__ in (
                    "InstPseudoReloadLibraryIndex",
                ):
                    _bass_rust.then_inc(inst, _lib_sems.pop(0), 16, False)
    mybir.codegen_inst_isa_subclasses(nc)

    _split_multi_waits(nc)
    return nc


def _build_program(mode: str) -> bass.Bass:
    if mode == "rescue":
        return _build_rescue_program()
    nc = bass.Bass()
    if mode == "split3":
        mm_dt = F16
        # hi+lo fp16 planes packed together: [g, p, plane, k, t]
        xb = nc.declare_dram_parameter("xb", [N_G, 128, 2, KC, T_G], F16,
                                       isOutput=False)
        wt = nc.declare_dram_parameter("wt", [128, KC, 2, E], F16,
                                       isOutput=False)
    else:
        mm_dt = F32R if mode == "f32r" else F32
        xb = nc.declare_dram_parameter("xb", [N_G, 128, KC, T_G], F32,
                                       isOutput=False)
        wt = nc.declare_dram_parameter("wt", [128, KC, E], F32, isOutput=False)
    w_out = nc.declare_dram_parameter("w_out", [T_CORE, TOPK], F32, isOutput=True)
    i_out = nc.declare_dram_parameter("i_out", [T_CORE, TOPK], I32, isOutput=True)

    def mmsrc(ap):
        return ap.bitcast(F32R) if mm_dt == F32R else ap

    with tile.TileContext(nc) as tc:
        with (
            tc.tile_pool(name="const", bufs=1) as const_pool,
            tc.tile_pool(name="xin", bufs=2) as xpool,
            tc.tile_pool(name="lsb", bufs=2) as lspool,
            tc.tile_pool(name="lg", bufs=4) as lgpool,
            tc.tile_pool(name="epi", bufs=4) as epool,
            tc.tile_pool(name="outg", bufs=2) as opool,
            tc.tile_pool(name="ps_l", bufs=2, space="PSUM") as ps_l,
            tc.tile_pool(name="ps_t", bufs=4, space="PSUM") as ps_t,
        ):
            ident = const_pool.tile([128, 128], F32)
            masks.make_identity(nc, ident[:])

            if mode == "split3":
                wt_sb = const_pool.tile([128, KC, 2, E], F16)
            else:
                wt_sb = const_pool.tile([128, KC, E], mm_dt)
            nc.sync.dma_start(wt_sb[:], mmsrc(wt[:]))

            for g in range(N_G):
                if mode == "split3":
                    xg = xpool.tile([128, 2, KC, T_G], F16, tag="xg")
                else:
                    xg = xpool.tile([128, KC, T_G], mm_dt, tag="xg")
                nc.sync.dma_start(xg[:], mmsrc(xb[g]))

                logitsT = ps_l.tile([E, T_G], F32, name="logitsT")
                if mode == "split3":
                    # x1@w1 + x1@w2 + x2@w1, accumulated in PSUM
                    terms = [(0, 0), (0, 1), (1, 0)]
                    n_mm = KC * len(terms)
                    i_mm = 0
                    for k in range(KC):
                        for (xi, wi) in terms:
                            nc.tensor.matmul(
                                logitsT[:],
                                wt_sb[:, k, wi, :],
                                xg[:, xi, k, :],
                                start=(i_mm == 0),
                                stop=(i_mm == n_mm - 1),
                            )
                            i_mm += 1
                else:
                    for k in range(KC):
                        nc.tensor.matmul(
                            logitsT[:],
                            wt_sb[:, k, :],
                            xg[:, k, :],
                            start=(k == 0),
                            stop=(k == KC - 1),
                        )

                ls = lspool.tile([E, T_G], F32, tag="ls")
                nc.scalar.copy(ls[:], logitsT[:])

                w_grp = opool.tile([128, T_G // 128, TOPK], F32, tag="wg")
                i_grp = opool.tile([128, T_G // 128, TOPK], I32, tag="ig")

                for j in range(T_G // 128):
                    lt_ps = ps_t.tile([128, E], F32, name="lt_ps")
                    nc.tensor.transpose(
                        lt_ps[:], ls[:, j * 128 : (j + 1) * 128], ident[:E, :E]
                    )
                    lg = lgpool.tile([128, E], F32, tag="lg")
                    nc.vector.tensor_copy(lg[:], lt_ps[:])

                    mx8 = epool.tile([128, TOPK], F32, tag="mx8")
                    nc.vector.max(mx8[:], lg[:])
                    nc.vector.max_index(
                        i_grp[:, j, :].bitcast(U32), mx8[:], lg[:]
                    )

                    negmax = epool.tile([128, 1], F32, tag="negmax")
                    nc.scalar.mul(negmax[:], mx8[:, 0:1], -1.0)

                    expall = epool.tile([128, E], F32, tag="expall")
                    denom = epool.tile([128, 1], F32, tag="denom")
                    nc.scalar.activation(
                        expall[:],
                        lg[:],
                        mybir.ActivationFunctionType.Exp,
                        bias=negmax[:],
                        accum_out=denom[:],
                    )
                    exp8 = epool.tile([128, TOPK], F32, tag="exp8")
                    nc.scalar.activation(
                        exp8[:],
                        mx8[:],
                        mybir.ActivationFunctionType.Exp,
                        bias=negmax[:],
                    )
                    r25 = epool.tile([128, 1], F32, tag="r25")
                    nc.vector.reciprocal(r25[:], denom[:])
                    nc.scalar.mul(r25[:], r25[:], ROUTE_SCALE)
                    nc.vector.tensor_scalar_mul(w_grp[:, j, :], exp8[:], r25[:])

                # outputs are tiny; keep them off the x-load DMA ring (HWDGE
                # has two physical rings — sync/SP carries the 8 MiB loads,
                # scalar/ACT carries the stores)
                nc.scalar.dma_start(
                    w_out[g * T_G : (g + 1) * T_G, :].rearrange(
                        "(j p) e -> p j e", p=128
                    ),
                    w_grp[:],
                )
                nc.scalar.dma_start(
                    i_out[g * T_G : (g + 1) * T_G, :].rearrange(
                        "(j p) e -> p j e", p=128
                    ),
                    i_grp[:],
                )

    _split_multi_waits(nc)
    return nc


_NC = {}


def _get_program(mode: str) -> bass.Bass:
    if mode not in _NC:
        _NC[mode] = _build_program(mode)
    return _NC[mode]


def _pack_x_core(shard: np.ndarray) -> np.ndarray:
    """[T_CORE, D] f32 -> [N_G, 128, KC, T_G] with
    xb[g, p, k, t] = shard[g*T_G + t, k*128 + p]."""
    v = shard.reshape(N_G, T_G, KC, 128)
    return np.ascontiguousarray(v.transpose(0, 3, 2, 1))


def _pack_w(W: np.ndarray) -> np.ndarray:
    """[E, D] -> [128, KC, E] with wt[p, k, e] = W[e, k*128 + p]."""
    v = W.reshape(E, KC, 128)
    return np.ascontiguousarray(v.transpose(2, 1, 0))


def _split16(a: np.ndarray):
    hi = a.astype(np.float16)
    lo = (a - hi.astype(np.float32)).astype(np.float16)
    return hi, lo


def _run(x: np.ndarray, W: np.ndarray, **kwargs):
    x = np.asarray(x, dtype=np.float32)
    W = np.asarray(W, dtype=np.float32)
    assert x.shape == (TOKENS, D), x.shape
    assert W.shape == (E, D), W.shape

    mode = MODE
    in_maps = []
    if mode == "rescue":
        w_pack = _pack_w(W)
        w1, w2 = _split16(w_pack)
        wt_host = np.ascontiguousarray(np.stack([w1, w2], axis=2))
        for c in range(N_CORES):
            shard = x[c * T_CORE : (c + 1) * T_CORE, :]
            hi = shard.astype(np.float16)
            lo = (shard - hi.astype(np.float32)).astype(np.float16)
            xb = np.ascontiguousarray(
                hi.reshape(N_G1, T_G1, KC, 128).transpose(0, 3, 2, 1)
            )
            xrows = np.ascontiguousarray(
                np.concatenate([hi, lo], axis=1)
            )  # [T_CORE, 2*D] f16
            in_maps.append({"xb": xb, "xrows": xrows, "wt": wt_host})
    elif mode == "split3":
        w_pack = _pack_w(W)  # [128, KC, E] f32
        w1, w2 = _split16(w_pack)
        wt_host = np.ascontiguousarray(
            np.stack([w1, w2], axis=2)
        )  # [128, KC, 2, E] f16
        for c in range(N_CORES):
            xp = _pack_x_core(x[c * T_CORE : (c + 1) * T_CORE, :])
            x1, x2 = _split16(xp)
            xb = np.ascontiguousarray(
                np.stack([x1, x2], axis=2)
            )  # [N_G, 128, 2, KC, T_G] f16
            in_maps.append({"xb": xb, "wt": wt_host})
    else:
        wt_host = _pack_w(W)
        for c in range(N_CORES):
            xb = _pack_x_core(x[c * T_CORE : (c + 1) * T_CORE, :])
            in_maps.append({"xb": xb, "wt": wt_host})

    nc = _get_program(mode)
    res = run_bass_kernel_spmd(nc, in_maps, core_ids=list(range(N_CORES)), **kwargs)

    weights = np.concatenate([res.results[c]["w_out"] for c in range(N_CORES)], axis=0)
    indices = np.concatenate([res.results[c]["i_out"] for c in range(N_CORES)], axis=0)
    return weights.astype(np.float32), indices.astype(np.int32), res


def kernel(x: np.ndarray, W: np.ndarray):
    weights, indices, _ = _run(x, W)
    return weights, indices



# revision 33
# speedup vs baseline: 1.2673x; 1.2673x over previous
"""MoE routing (gate) kernel for Trainium2, 8 NeuronCores, data-parallel.

Computes, for x [65536, 4096] f32 and W [64, 4096] f32:
    logits  = x @ W.T                       # [65536, 64]
    scores  = softmax(logits, axis=-1)
    weights, indices = top_k(scores, 8)     # [65536, 8] each
    weights *= 2.5

Sharding: token dim split 8 ways (8192 tokens/core); W replicated.

Host-side prep packs each core's x shard into the exact SBUF tile
layout so every group load is ONE fully contiguous 8 MiB DMA:
    xb[g, p, k, t] = x[g*T_G + t, k*128 + p]   (per core shard)
and W into wt[p, k, e] = W[e, k*128 + p].

Per-core program (Tile framework), for each group of 512 tokens:
  - one 8 MiB contiguous dma_start -> xg [128, KC, T_G]
  - KC accumulating PE matmuls: logitsT[64, T_G] += wt_k.T @ xg_k
  - copy PSUM->SBUF, 4 PE transposes -> logits [128 tok, 64 exp]
  - DVE max/max_index -> top-8 values + indices (desc order, first-index
    tie-break = jax.lax.top_k order)
  - ACT exp(x - max) with accumulated row-sum -> softmax denominator
  - weights = exp(top8 - max) * 2.5 / denom

Matmul dtype modes (GATE_MODE):
  f32r   - fp32 data streamed as float32r: 1 cyc/row (4x faster than f32)
  split3 - x,W pre-split host-side into fp16 hi+lo planes; logits =
           x1@w1 + x1@w2 + x2@w1 (3 bf16-rate passes, near-fp32 exact)
  f32    - plain fp32 (4 cyc/row), exact
"""

import os
import sys

for _p in ("/opt/trn_rl_repo", "/root/.axon_site/_ro/trn_rl_repo"):
    if os.path.isdir(_p) and _p not in sys.path:
        sys.path.append(_p)

import numpy as np

import concourse.bass as bass
import concourse.mybir as mybir
from concourse import library_config, masks, tile
from concourse.bass_utils import run_bass_kernel_spmd
from concourse.vector_clock import ScopedClock

TOKENS = 65536
D = 4096
E = 64
TOPK = 8
ROUTE_SCALE = 2.5
N_CORES = 8
T_CORE = TOKENS // N_CORES  # 8192
T_G = 512                   # tokens per group (one PSUM bank at fp32)
N_G = T_CORE // T_G         # 16
KC = D // 128               # 32 contraction chunks

F32 = mybir.dt.float32
F32R = mybir.dt.float32r
F16 = mybir.dt.float16
I32 = mybir.dt.int32
U32 = mybir.dt.uint32

MODE = os.environ.get("GATE_MODE", "split3")  # v2 | split3 | rescue | f32r | f32

# --- v2-mode constants ------------------------------------------------------
V2_THRESH = 1.0e-3          # min-gap flag threshold (~4 sigma of x1@W error)
V2_R_PP = 8                 # rescue slots per partition (capacity 1024)
V2_R_CHUNK = 512            # rescued tokens per gather/matmul chunk
V2_N_RCH = 128 * V2_R_PP // V2_R_CHUNK   # 2 chunks
V2_S_PER_CH = V2_R_CHUNK // 128          # 4 slots per chunk
V2_PHASE = os.environ.get("GATE_V2_PHASE", "full")  # full | A

# --- rescue-mode constants -------------------------------------------------
T_G1 = 512                  # tokens per approx group (one PSUM bank at fp32)
N_G1 = T_CORE // T_G1       # 16
THRESH = 3.0e-3             # min-gap flag threshold (~4 sigma of fp16 error)
R_PP = 16                   # rescue slots per partition (capacity 2048)
R_CHUNK = 512               # rescued tokens per gather/matmul chunk
N_RCH = 128 * R_PP // R_CHUNK  # 4 chunks
S_PER_CH = R_CHUNK // 128   # 4 slots per chunk

# ---------------------------------------------------------------------------
# Walrus in this container rejects >1 sync-wait on control instructions; the
# stock TileContext tail drain carries one wait per live processor.  Spread
# them across sync-engine NOPs (1 each) before the drain.
_MAX_WAITS = 1


def _patched_drain_and_barrier(self, tick_clock, wait_clock):
    nc = self.nc
    probe = nc.sync.nop()
    wait_clock.add_sem_waits(probe.ins, ScopedClock({None: tick_clock.global_clock}))
    waits = list(probe.ins.sync_info.on_wait or [])
    probe.ins.sync_info.on_wait = waits[:_MAX_WAITS]
    for i in range(_MAX_WAITS, len(waits), _MAX_WAITS):
        extra = nc.sync.nop()
        if extra.ins.sync_info is None:
            extra.ins.sync_info = mybir.SyncInfo(
                on_wait=waits[i : i + _MAX_WAITS], on_update=[]
            )
        else:
            extra.ins.sync_info.on_wait = waits[i : i + _MAX_WAITS]
    nc.sync.drain()

    nc.all_engine_barrier()
    assert self.sems is not None
    popped = nc._tile_sem_poison_stack.pop()
    assert popped is self._sem_poison
    nc.clear_and_free_semaphores(list(self.sems.allocated().values()))
    nc.all_engine_barrier()


tile.TileContext._drain_and_barrier = _patched_drain_and_barrier


def _split_multi_waits(nc: bass.Bass, max_waits: int = _MAX_WAITS):
    """Walrus here caps sync waits at 1 per instruction (any engine struct).
    Hoist excess waits onto same-engine NOPs inserted just before the
    offending instruction — the sequencer satisfies them in order, so the
    semantics (AND of all waits before execute) are preserved."""
    n = 0
    for fn in nc.m.functions:
        for bb in fn.blocks:
            out = []
            changed = False
            for inst in bb.instructions:
                si = inst.sync_info
                w = list(si.on_wait) if (si and si.on_wait) else []
                if len(w) > max_waits:
                    extras = w[: len(w) - max_waits]
                    si.on_wait = w[len(w) - max_waits :]
                    for i0 in range(0, len(extras), max_waits):
                        nop = mybir.InstNoOp(
                            name=f"I-wsplit-{nc.next_id()}", ins=[], outs=[]
                        )
                        nop.engine = inst.engine
                        nop.sync_info = mybir.SyncInfo(
                            on_wait=extras[i0 : i0 + max_waits], on_update=[]
                        )
                        out.append(nop)
                        n += 1
                    changed = True
                out.append(inst)
            if changed:
                bb.instructions = out
    return n
# ---------------------------------------------------------------------------


def _build_rescue_program() -> bass.Bass:
    """fp16 approx pass + exact rescue of near-tie tokens.

    Phase A: logits ~= x1 @ w1 (hi fp16 planes), full top-8 epilogue for all
      tokens, plus a per-token flag = (min adjacent gap of ranks 1..9 < t).
    Phase B: per-partition compaction of flags into <=R_PP rescue slots;
      token id of slot (p, s) = slot_value*128 + p.
    Phase C: dma_gather(transpose) pulls the flagged tokens' full-precision
      rows (hi|lo fp16), recomputes exact logits (3-term split), redoes the
      epilogue, and indirect-scatters weights/indices over the approx rows.
    """
    nc = bass.Bass()
    xb = nc.declare_dram_parameter("xb", [N_G1, 128, KC, T_G1], F16, isOutput=False)
    xrows = nc.declare_dram_parameter("xrows", [T_CORE, 2 * D], F16, isOutput=False)
    wt = nc.declare_dram_parameter("wt", [128, KC, 2, E], F16, isOutput=False)
    w_out = nc.declare_dram_parameter("w_out", [T_CORE, TOPK], F32, isOutput=True)
    i_out = nc.declare_dram_parameter("i_out", [T_CORE, TOPK], I32, isOutput=True)

    I16 = mybir.dt.int16
    ALU = mybir.AluOpType

    with tile.TileContext(nc) as tc:
        st_sem = nc.alloc_semaphore(name="st_done")
        idx_sem = nc.alloc_semaphore(name="idx_done")
        g_sem = nc.alloc_semaphore(name="g_done")
        sc_sem = nc.alloc_semaphore(name="sc_done")
        lib0 = nc.alloc_semaphore(name="lib_ld0")
        lib1 = nc.alloc_semaphore(name="lib_ld1")
        nc._gate_lib_sems = [lib0, lib1]
        _my_sems = [st_sem, idx_sem, g_sem, sc_sem, lib0, lib1]
        with (
            tc.tile_pool(name="const", bufs=1) as const_pool,
            tc.tile_pool(name="state", bufs=1) as state_pool,
            tc.tile_pool(name="lsb", bufs=2) as lspool,
            tc.tile_pool(name="lg", bufs=4) as lgpool,
            tc.tile_pool(name="epi", bufs=6) as epool,
            tc.tile_pool(name="outg", bufs=2) as opool,
            tc.tile_pool(name="ps_t", bufs=2, space="PSUM") as ps_t,
            tc.tile_pool(name="ps_i", bufs=1, space="PSUM") as ps_i,
        ):
            ident = const_pool.tile([128, 128], F32)
            masks.make_identity(nc, ident[:])
            wt_sb = const_pool.tile([128, KC, 2, E], F16)
            nc.sync.dma_start(wt_sb[:], wt[:])
            # pm128[p] = p - 128 (for slot-value -> token-id arithmetic)
            pm128_i = const_pool.tile([128, 1], I32)
            nc.gpsimd.iota(pm128_i[:], pattern=[[0, 1]], base=-128,
                           channel_multiplier=1)
            pm128 = const_pool.tile([128, 1], F32)
            nc.vector.tensor_copy(pm128[:], pm128_i[:])

            fscore = state_pool.tile([128, 64], F32)
            ids_f = state_pool.tile([128, R_PP], F32)       # token ids (pad<0)
            ids_scat = state_pool.tile([128, R_PP], I32)    # pad -> OOB large
            idx_chunks = [
                state_pool.tile([128, R_CHUNK // 16], I16, name=f"idxc{c}")
                for c in range(N_RCH)
            ]

            def epilogue_tile(ls, j, w_grp, i_grp, flag_slot=None):
                """One 128-token tile: transpose, top-8, softmax; optionally
                write the near-tie flag score into fscore[:, flag_slot]."""
                lt_ps = ps_t.tile([128, E], F32, name="lt_ps")
                nc.tensor.transpose(
                    lt_ps[:], ls[:, j * 128 : (j + 1) * 128], ident[:E, :E]
                )
                lg = lgpool.tile([128, E], F32, tag="lg")
                nc.vector.tensor_copy(lg[:], lt_ps[:])

                mx8 = epool.tile([128, TOPK], F32, tag="mx8")
                nc.vector.max(mx8[:], lg[:])
                nc.vector.max_index(i_grp[:, j, :].bitcast(U32), mx8[:], lg[:])

                if flag_slot is not None:
                    lgrm = lgpool.tile([128, E], F32, tag="lgrm")
                    nc.vector.match_replace(lgrm[:], mx8[:], lg[:], -1e30)
                    mx9 = epool.tile([128, TOPK], F32, tag="mx9")
                    nc.vector.max(mx9[:], lgrm[:])
                    t9 = epool.tile([128, 9], F32, tag="t9")
                    nc.vector.tensor_copy(t9[:, 0:8], mx8[:])
                    nc.vector.tensor_copy(t9[:, 8:9], mx9[:, 0:1])
                    # neg-gaps: rank_{k+1} - rank_k; max8 head = -min(gap)
                    nggaps = epool.tile([128, 8], F32, tag="nggaps")
                    nc.vector.tensor_tensor(
                        out=nggaps[:], in0=t9[:, 1:9], in1=t9[:, 0:8],
                        op=ALU.subtract,
                    )
                    mxg = epool.tile([128, 8], F32, tag="mxg")
                    nc.vector.max(mxg[:], nggaps[:])
                    # fscore[:, s] = (min gap < t) * (s + 1)
                    nc.vector.tensor_scalar(
                        out=fscore[:, flag_slot : flag_slot + 1],
                        in0=mxg[:, 0:1],
                        scalar1=-THRESH,
                        scalar2=float(flag_slot + 1),
                        op0=ALU.is_gt,
                        op1=ALU.mult,
                    )

                negmax = epool.tile([128, 1], F32, tag="negmax")
                nc.scalar.mul(negmax[:], mx8[:, 0:1], -1.0)
                expall = epool.tile([128, E], F32, tag="expall")
                denom = epool.tile([128, 1], F32, tag="denom")
                nc.scalar.activation(
                    expall[:],
                    lg[:],
                    mybir.ActivationFunctionType.Exp,
                    bias=negmax[:],
                    accum_out=denom[:],
                )
                exp8 = epool.tile([128, TOPK], F32, tag="exp8")
                nc.scalar.activation(
                    exp8[:],
                    mx8[:],
                    mybir.ActivationFunctionType.Exp,
                    bias=negmax[:],
                )
                r25 = epool.tile([128, 1], F32, tag="r25")
                nc.vector.reciprocal(r25[:], denom[:])
                nc.scalar.mul(r25[:], r25[:], ROUTE_SCALE)
                nc.vector.tensor_scalar_mul(w_grp[:, j, :], exp8[:], r25[:])

            # ---------------- Phase A: approx pass -------------------------
            with (
                tc.tile_pool(name="xin", bufs=2) as xpool,
                tc.tile_pool(name="ps_l", bufs=2, space="PSUM") as ps_l,
            ):
                for g in range(N_G1):
                    xg = xpool.tile([128, KC, T_G1], F16, tag="xg")
                    nc.sync.dma_start(xg[:], xb[g])
                    logitsT = ps_l.tile([E, T_G1], F32, name="logitsT")
                    for k in range(KC):
                        nc.tensor.matmul(
                            logitsT[:],
                            wt_sb[:, k, 0, :],
                            xg[:, k, :],
                            start=(k == 0),
                            stop=(k == KC - 1),
                        )
                    ls = lspool.tile([E, T_G1], F32, tag="ls")
                    nc.scalar.copy(ls[:], logitsT[:])

                    w_grp = opool.tile([128, T_G1 // 128, TOPK], F32, tag="wg")
                    i_grp = opool.tile([128, T_G1 // 128, TOPK], I32, tag="ig")
                    for j in range(T_G1 // 128):
                        epilogue_tile(
                            ls, j, w_grp, i_grp,
                            flag_slot=g * (T_G1 // 128) + j,
                        )

                    nc.scalar.dma_start(
                        w_out[g * T_G1 : (g + 1) * T_G1, :].rearrange(
                            "(j p) e -> p j e", p=128
                        ),
                        w_grp[:],
                    ).then_inc(st_sem, 16)
                    nc.scalar.dma_start(
                        i_out[g * T_G1 : (g + 1) * T_G1, :].rearrange(
                            "(j p) e -> p j e", p=128
                        ),
                        i_grp[:],
                    ).then_inc(st_sem, 16)

            # ---------------- Phase B: compaction --------------------------
            m0 = state_pool.tile([128, 8], F32)
            nc.vector.max(m0[:], fscore[:])
            f1 = state_pool.tile([128, 64], F32)
            nc.vector.match_replace(f1[:], m0[:], fscore[:], 0.0)
            m1 = state_pool.tile([128, 8], F32)
            nc.vector.max(m1[:], f1[:])
            vals = state_pool.tile([128, R_PP], F32)
            nc.vector.tensor_copy(vals[:, 0:8], m0[:])
            nc.vector.tensor_copy(vals[:, 8:16], m1[:])

            # ids = vals*128 + (p-128); pad (vals==0) -> p-128 < 0
            nc.vector.tensor_scalar(
                out=ids_f[:], in0=vals[:], scalar1=128.0, scalar2=pm128[:, 0:1],
                op0=ALU.mult, op1=ALU.add,
            )
            # gather ids: pad -> 0 (valid dummy row; result discarded)
            idg_f = state_pool.tile([128, R_PP], F32)
            isv = state_pool.tile([128, R_PP], F32)
            nc.vector.tensor_scalar(
                out=isv[:], in0=vals[:], scalar1=0.0, scalar2=None,
                op0=ALU.is_gt,
            )
            nc.vector.tensor_tensor(
                out=idg_f[:], in0=ids_f[:], in1=isv[:], op=ALU.mult
            )
            # scatter ids: pad -> +20000+ (past bounds_check -> skipped)
            pad_of = state_pool.tile([128, R_PP], F32)
            nc.vector.tensor_scalar(
                out=pad_of[:], in0=vals[:], scalar1=0.0, scalar2=20128.0,
                op0=ALU.is_le, op1=ALU.mult,
            )
            ids_scat_f = state_pool.tile([128, R_PP], F32)
            nc.vector.tensor_tensor(
                out=ids_scat_f[:], in0=ids_f[:], in1=pad_of[:], op=ALU.add
            )
            nc.vector.tensor_copy(ids_scat[:], ids_scat_f[:])

            # transpose gather ids to [16, 128] wrap via PE
            idsT_ps = ps_i.tile([R_PP, 128], F32, name="idsT")
            nc.tensor.transpose(idsT_ps[:], idg_f[:], ident[:, :])
            idsT_f = state_pool.tile([R_PP, 128], F32)
            nc.vector.tensor_copy(idsT_f[:], idsT_ps[:])
            idsT_i16 = state_pool.tile([R_PP, 128], I16)
            nc.vector.tensor_copy(idsT_i16[:], idsT_f[:])

            # build per-chunk replicated idx tiles [128, R_CHUNK//16]:
            # chunk slot i = s_local*128 + p lives at [i % 16, i // 16]
            # (interleaved wrap, per bass_interp InstDMAGatherAnt).
            # SBUF APs cannot turn a free-dim stride into a partition step, so
            # bounce through a DRAM scratch where arbitrary APs are legal.
            ids_dram = nc.dram_tensor(
                "ids_scratch", [R_PP, 128], I16, kind="Internal"
            )
            nc.sync.dma_start(ids_dram[:], idsT_i16[:]).then_inc(idx_sem, 16)
            nc.sync.wait_ge(idx_sem, 16)
            for c in range(N_RCH):
                idxc = idx_chunks[c]
                src = ids_dram[c * S_PER_CH : (c + 1) * S_PER_CH, :].rearrange(
                    "s (a q) -> q (s a)", a=8
                )
                nc.sync.dma_start(idxc[0:16, :], src).then_inc(idx_sem, 16)
                nc.sync.dma_start(idxc[16:32, :], idxc[0:16, :]).then_inc(
                    idx_sem, 16
                )
                nc.sync.dma_start(idxc[32:64, :], idxc[0:32, :]).then_inc(
                    idx_sem, 16
                )
                nc.sync.dma_start(idxc[64:128, :], idxc[0:64, :]).then_inc(
                    idx_sem, 16
                )

            # ---------------- Phase C: exact rescue ------------------------
            with (
                tc.tile_pool(name="xres", bufs=2) as gpool,
                tc.tile_pool(name="ps_r", bufs=2, space="PSUM") as ps_r,
            ):
                first_scatter = True
                for c in range(N_RCH):
                    xr = gpool.tile([128, 2 * KC, R_CHUNK], F16, tag="xr")
                    if c == 0:
                        # all 16 idx-build DMAs + the ids_dram store landed
                        nc.gpsimd.wait_ge(idx_sem, 16 * 17)
                    nc.gpsimd.dma_gather(
                        xr[:], xrows[:], idx_chunks[c][:],
                        R_CHUNK, R_CHUNK, 2 * D, transpose=True,
                    ).then_inc(g_sem, 16)
                    # Tile treats the gather as instruction-retire complete;
                    # the matmuls must wait for the DATA to land.
                    nc.tensor.wait_ge(g_sem, 16 * (c + 1))
                    r_ps = ps_r.tile([E, R_CHUNK], F32, name="r_ps")
                    i_mm = 0
                    for k in range(KC):
                        for (xi, wi) in ((0, 0), (0, 1), (1, 0)):
                            nc.tensor.matmul(
                                r_ps[:],
                                wt_sb[:, k, wi, :],
                                xr[:, xi * KC + k, :],
                                start=(i_mm == 0),
                                stop=(i_mm == 3 * KC - 1),
                            )
                            i_mm += 1
                    rls = lspool.tile([E, R_CHUNK], F32, tag="rls")
                    nc.scalar.copy(rls[:], r_ps[:])

                    rw = opool.tile([128, S_PER_CH, TOPK], F32, tag="rw")
                    ri = opool.tile([128, S_PER_CH, TOPK], I32, tag="ri")
                    for j in range(S_PER_CH):
                        epilogue_tile(rls, j, rw, ri, flag_slot=None)

                    if first_scatter:
                        nc.gpsimd.wait_ge(st_sem, 16 * 2 * N_G1)
                        first_scatter = False
                    for j in range(S_PER_CH):
                        s = c * S_PER_CH + j
                        nc.gpsimd.indirect_dma_start(
                            out=w_out[:],
                            out_offset=bass.IndirectOffsetOnAxis(
                                ap=ids_scat[:, s : s + 1], axis=0
                            ),
                            in_=rw[:, j, :],
                            in_offset=None,
                            bounds_check=T_CORE - 1,
                            oob_is_err=False,
                        ).then_inc(sc_sem, 16)
                        nc.gpsimd.indirect_dma_start(
                            out=i_out[:],
                            out_offset=bass.IndirectOffsetOnAxis(
                                ap=ids_scat[:, s : s + 1], axis=0
                            ),
                            in_=ri[:, j, :],
                            in_offset=None,
                            bounds_check=T_CORE - 1,
                            oob_is_err=False,
                        ).then_inc(sc_sem, 16)

                # all rescue scatters must land before the kernel drains
                nc.gpsimd.wait_ge(sc_sem, 16 * 2 * N_RCH * S_PER_CH)
                del _my_sems  # left set at end; runtime zeroes sems per load

    # Raw Bass skips Bacc's compile passes; gpsimd extended insts
    # (dma_gather) need the GPSIMD library loads inserted and the ISA
    # bytes generated, else walrus dies with "ISA wrong length".
    import bass_rust as _bass_rust
    from concourse.library_config import all_libraries, standard

    inst_type_to_lib_mask = {}
    for lib in all_libraries:
        for inst_type in lib.instructions:
            inst_type_to_lib_mask[inst_type] = inst_type_to_lib_mask.get(
                inst_type, 0
            ) | (1 << lib.index)
    _bass_rust.insert_library_loads(
        nc, inst_type_to_lib_mask, len(all_libraries), standard.index
    )
    # The inserted library loads are DMA-classified (ucode fetch) but carry
    # no sem update, which the race detector / BIR invariants reject. Give
    # them a dedicated sem nobody waits on.
    _lib_sems = list(nc._gate_lib_sems)
    for fn in nc.m.functions:
        for bb in fn.blocks:
            for inst in bb.instructions:
                if type(inst).__name__ in (
                    "InstPseudoReloadLibraryIndex",
                ):
                    _bass_rust.then_inc(inst, _lib_sems.pop(0), 16, False)
    mybir.codegen_inst_isa_subclasses(nc)

    _split_multi_waits(nc)
    return nc


def _build_v2_program() -> bass.Bass:
    """fp16 hi-plane matmul with BOTH W planes packed into the 128 PE
    columns + exact rescue of near-tie tokens via dma_gather; rescue
    results land in a dense side buffer that the host merges (no
    indirect scatter).

    Phase A: one PSUM [128, T_G] per group: partitions 0-63 = x1@w1,
      64-127 = x1@w2 (lhsT = [w1|w2], 128 cols). DVE adds the halves ->
      approx logits = x1@W (exact-W precision, sigma ~2.7e-4). Full
      top-8 epilogue for all tokens + per-token near-tie flag
      (min adjacent gap of approx top-9 < V2_THRESH).
    Phase B: per-partition compaction of flags into <=V2_R_PP slots;
      token id of slot (p, s) = slot_value*128 + p.
    Phase C: dma_gather(transpose) pulls flagged tokens' [x1|x2] f16
      rows; exact logits via 2-pass packed matmul (x1@[w1|w2] then
      x2@[w1|w2], all four split terms accumulated); epilogue; results
      stored DENSELY to rescue_w/rescue_i/rescue_id DRAM for host merge.
    """
    nc = bass.Bass()
    xb = nc.declare_dram_parameter("xb", [N_G, 128, KC, T_G], F16, isOutput=False)
    xrows = nc.declare_dram_parameter("xrows", [T_CORE, 2 * D], F16, isOutput=False)
    wt = nc.declare_dram_parameter("wt", [128, KC, 2 * E], F16, isOutput=False)
    w_out = nc.declare_dram_parameter("w_out", [T_CORE, TOPK], F32, isOutput=True)
    i_out = nc.declare_dram_parameter("i_out", [T_CORE, TOPK], I32, isOutput=True)
    resc_w = nc.declare_dram_parameter(
        "resc_w", [128, V2_N_RCH, V2_S_PER_CH, TOPK], F32, isOutput=True
    )
    resc_i = nc.declare_dram_parameter(
        "resc_i", [128, V2_N_RCH, V2_S_PER_CH, TOPK], I32, isOutput=True
    )
    resc_id = nc.declare_dram_parameter(
        "resc_id", [128, V2_R_PP], I32, isOutput=True
    )
    I16 = mybir.dt.int16

    ALU = mybir.AluOpType
    phase = V2_PHASE
    do_idx = phase in ("B", "AB", "full")
    do_gather = phase in ("AB", "full")
    do_rescue = phase == "full"

    with tile.TileContext(nc) as tc:
        g_sem = nc.alloc_semaphore(name="g_done")
        lib0 = nc.alloc_semaphore(name="lib_ld0")
        lib1 = nc.alloc_semaphore(name="lib_ld1")
        nc._gate_lib_sems = [lib0, lib1]
        with (
            tc.tile_pool(name="const", bufs=1) as const_pool,
            tc.tile_pool(name="state", bufs=1) as state_pool,
            tc.tile_pool(name="lsb", bufs=2) as lspool,
            tc.tile_pool(name="lg", bufs=4) as lgpool,
            tc.tile_pool(name="epi", bufs=6) as epool,
            tc.tile_pool(name="outg", bufs=2) as opool,
            tc.tile_pool(name="ps_t", bufs=2, space="PSUM") as ps_t,
            tc.tile_pool(name="ps_i", bufs=1, space="PSUM") as ps_i,
        ):
            ident = const_pool.tile([128, 128], F32)
            masks.make_identity(nc, ident[:])
            wt_sb = const_pool.tile([128, KC, 2 * E], F16)
            nc.sync.dma_start(wt_sb[:], wt[:])
            # pm128[p] = p - 128 (for slot-value -> token-id arithmetic)
            pm128_i = const_pool.tile([128, 1], I32)
            nc.gpsimd.iota(pm128_i[:], pattern=[[0, 1]], base=-128,
                           channel_multiplier=1)
            pm128 = const_pool.tile([128, 1], F32)
            nc.vector.tensor_copy(pm128[:], pm128_i[:])

            fscore = state_pool.tile([128, 64], F32)
            idx_chunks = [
                state_pool.tile([128, V2_R_CHUNK // 16], I16, name=f"idxc{c}")
                for c in range(V2_N_RCH)
            ]

            def epilogue_tile(ls, j, w_grp, i_grp, flag_slot=None):
                """One 128-token tile from the full-width [128, T] logits
                (PSUM-partition halves = x@w1 / x@w2): PE transpose to
                token-major, DVE free-dim add of the halves, then top-8 +
                softmax; optionally write the near-tie flag score into
                fscore[:, flag_slot]."""
                lt_ps = ps_t.tile([128, 128], F32, name="lt_ps")
                nc.tensor.transpose(
                    lt_ps[:], ls[:, j * 128 : (j + 1) * 128], ident[:, :]
                )
                lgw = lgpool.tile([128, 128], F32, tag="lgw")
                nc.vector.tensor_copy(lgw[:], lt_ps[:])
                # logits = x@w1 + x@w2 (same lanes, free-dim halves)
                lg = lgpool.tile([128, E], F32, tag="lg")
                nc.vector.tensor_tensor(
                    out=lg[:], in0=lgw[:, 0:E], in1=lgw[:, E:128],
                    op=ALU.add,
                )

                mx8 = epool.tile([128, TOPK], F32, tag="mx8")
                nc.vector.max(mx8[:], lg[:])
                nc.vector.max_index(i_grp[:, j, :].bitcast(U32), mx8[:], lg[:])

                if flag_slot is not None:
                    lgrm = lgpool.tile([128, E], F32, tag="lgrm")
                    nc.vector.match_replace(lgrm[:], mx8[:], lg[:], -1e30)
                    mx9 = epool.tile([128, TOPK], F32, tag="mx9")
                    nc.vector.max(mx9[:], lgrm[:])
                    t9 = epool.tile([128, 9], F32, tag="t9")
                    nc.vector.tensor_copy(t9[:, 0:8], mx8[:])
                    nc.vector.tensor_copy(t9[:, 8:9], mx9[:, 0:1])
                    # neg-gaps: rank_{k+1} - rank_k; max8 head = -min(gap)
                    nggaps = epool.tile([128, 8], F32, tag="nggaps")
                    nc.vector.tensor_tensor(
                        out=nggaps[:], in0=t9[:, 1:9], in1=t9[:, 0:8],
                        op=ALU.subtract,
                    )
                    mxg = epool.tile([128, 8], F32, tag="mxg")
                    nc.vector.max(mxg[:], nggaps[:])
                    # fscore[:, s] = (min gap < t) * (s + 1)
                    nc.vector.tensor_scalar(
                        out=fscore[:, flag_slot : flag_slot + 1],
                        in0=mxg[:, 0:1],
                        scalar1=-V2_THRESH,
                        scalar2=float(flag_slot + 1),
                        op0=ALU.is_gt,
                        op1=ALU.mult,
                    )

                negmax = epool.tile([128, 1], F32, tag="negmax")
                nc.scalar.mul(negmax[:], mx8[:, 0:1], -1.0)
                expall = epool.tile([128, E], F32, tag="expall")
                denom = epool.tile([128, 1], F32, tag="denom")
                nc.scalar.activation(
                    expall[:],
                    lg[:],
                    mybir.ActivationFunctionType.Exp,
                    bias=negmax[:],
                    accum_out=denom[:],
                )
                exp8 = epool.tile([128, TOPK], F32, tag="exp8")
                nc.scalar.activation(
                    exp8[:],
                    mx8[:],
                    mybir.ActivationFunctionType.Exp,
                    bias=negmax[:],
                )
                r25 = epool.tile([128, 1], F32, tag="r25")
                nc.vector.reciprocal(r25[:], denom[:])
                nc.scalar.mul(r25[:], r25[:], ROUTE_SCALE)
                nc.vector.tensor_scalar_mul(w_grp[:, j, :], exp8[:], r25[:])

            # ---------------- Phase A: approx pass -------------------------
            with (
                tc.tile_pool(name="xin", bufs=2) as xpool,
                tc.tile_pool(name="ps_l", bufs=2, space="PSUM") as ps_l,
            ):
                for g in range(N_G):
                    xg = xpool.tile([128, KC, T_G], F16, tag="xg")
                    nc.sync.dma_start(xg[:], xb[g])
                    lps = ps_l.tile([128, T_G], F32, name="lps")
                    for k in range(KC):
                        nc.tensor.matmul(
                            lps[:],
                            wt_sb[:, k],
                            xg[:, k],
                            start=(k == 0),
                            stop=(k == KC - 1),
                        )
                    # full-width copy PSUM->SBUF; the per-tile transpose +
                    # free-dim add in epilogue_tile sums the w1/w2 halves
                    ls = lspool.tile([128, T_G], F32, tag="ls")
                    nc.scalar.copy(ls[:], lps[:])

                    w_grp = opool.tile([128, T_G // 128, TOPK], F32, tag="wg")
                    i_grp = opool.tile([128, T_G // 128, TOPK], I32, tag="ig")
                    for j in range(T_G // 128):
                        epilogue_tile(
                            ls, j, w_grp, i_grp,
                            flag_slot=g * (T_G // 128) + j,
                        )

                    nc.scalar.dma_start(
                        w_out[g * T_G : (g + 1) * T_G, :].rearrange(
                            "(j p) e -> p j e", p=128
                        ),
                        w_grp[:],
                    )
                    nc.scalar.dma_start(
                        i_out[g * T_G : (g + 1) * T_G, :].rearrange(
                            "(j p) e -> p j e", p=128
                        ),
                        i_grp[:],
                    )

            # ---------------- Phase B: compaction --------------------------
            # vals = top-8 fscores (desc) per partition = last 8 flagged slots
            vals = state_pool.tile([128, V2_R_PP], F32)
            nc.vector.max(vals[:], fscore[:])

            # ids = vals*128 + (p-128); pad (vals==0) -> p-128 < 0
            ids_f = state_pool.tile([128, V2_R_PP], F32)
            nc.vector.tensor_scalar(
                out=ids_f[:], in0=vals[:], scalar1=128.0, scalar2=pm128[:, 0:1],
                op0=ALU.mult, op1=ALU.add,
            )
            # host-merge id output (pad stays negative)
            ids_i = state_pool.tile([128, V2_R_PP], I32)
            nc.vector.tensor_copy(ids_i[:], ids_f[:])
            nc.scalar.dma_start(resc_id[:], ids_i[:])

            # gather ids: pad -> 0 (valid dummy row; result discarded)
            idg_f = state_pool.tile([128, V2_R_PP], F32)
            isv = state_pool.tile([128, V2_R_PP], F32)
            nc.vector.tensor_scalar(
                out=isv[:], in0=vals[:], scalar1=0.0, scalar2=None,
                op0=ALU.is_gt,
            )
            nc.vector.tensor_tensor(
                out=idg_f[:], in0=ids_f[:], in1=isv[:], op=ALU.mult
            )

            if do_idx:
                # Build per-chunk replicated idx tiles [128, R_CHUNK//16]:
                # chunk slot i = s_local*128 + p lives at [i % 16, i // 16]
                # (interleaved wrap, per bass_interp InstDMAGatherAnt), i.e.
                # idxc[q, s*8+a] = ids[a*16+q, s]. Built entirely on-chip
                # via a PE double-transpose (no DRAM bounce: Internal DRAM
                # is unallocated under bass2jax/PJRT, and manual DMA sems
                # collide with Tile's auto updates):
                #   T1[s, p] = ids[p, s]          (one [128,8] transpose)
                #   T2_a[q, s] = T1[s, a*16+q]    (8 tiny [8,16] transposes)
                # then strided DVE copies interleave T2_a into idxc.
                idsT_ps = ps_i.tile([V2_R_PP, 128], F32, name="idsT")
                nc.tensor.transpose(idsT_ps[:], idg_f[:], ident[:, :])
                idsT_f = state_pool.tile([V2_R_PP, 128], F32)
                nc.vector.tensor_copy(idsT_f[:], idsT_ps[:])

                idxc_f = [
                    state_pool.tile(
                        [16, 8 * V2_S_PER_CH], F32, name=f"idxf{c}"
                    )
                    for c in range(V2_N_RCH)
                ]
                for a in range(8):
                    t2 = ps_i.tile([16, V2_R_PP], F32, tag="t2")
                    nc.tensor.transpose(
                        t2[:],
                        idsT_f[:, a * 16 : (a + 1) * 16],
                        ident[:V2_R_PP, :V2_R_PP],
                    )
                    for c in range(V2_N_RCH):
                        # idxc_f[c][q, s*8+a] = t2[q, c*S+s]
                        dst = idxc_f[c][:].rearrange("q (s a) -> q s a", a=8)
                        nc.vector.tensor_copy(
                            dst[:, :, a],
                            t2[:, c * V2_S_PER_CH : (c + 1) * V2_S_PER_CH],
                        )
                for c in range(V2_N_RCH):
                    idxc = idx_chunks[c]
                    nc.vector.tensor_copy(idxc[0:16, :], idxc_f[c][:])
                    nc.sync.dma_start(idxc[16:32, :], idxc[0:16, :])
                    nc.sync.dma_start(idxc[32:64, :], idxc[0:32, :])
                    nc.sync.dma_start(idxc[64:128, :], idxc[0:64, :])

                # ---------------- Phase C: exact rescue --------------------
                if do_gather:
                  with (
                    tc.tile_pool(name="xres", bufs=2) as gpool,
                    tc.tile_pool(name="ps_r", bufs=2, space="PSUM") as ps_r,
                  ):
                    for c in range(V2_N_RCH):
                        # SWDGE descriptor ring caps a transpose gather at
                        # 128 rows/op (s2m ~4 descs/16KB row, ring 128
                        # entries) — a 512-row gather dies in ucode reclaim.
                        # Each 128-row sub-gather needs a contiguous output
                        # block, so the tile is [128, S, 2KC, 128] and the
                        # matmul reads the strided [128, S, 128] slice.
                        xr = gpool.tile(
                            [128, V2_S_PER_CH, 2 * KC, 128], F16, tag="xr"
                        )
                        for s in range(V2_S_PER_CH):
                            nc.gpsimd.dma_gather(
                                xr[:, s],
                                xrows[:],
                                idx_chunks[c][:, s * 8 : (s + 1) * 8],
                                128, 128, 2 * D, transpose=True,
                            ).then_inc(g_sem, 16)
                        # Tile treats the gather as instruction-retire
                        # complete; the matmuls must wait for the DATA.
                        if not do_rescue:
                            continue
                        nc.tensor.wait_ge(
                            g_sem, 16 * V2_S_PER_CH * (c + 1)
                        )
                        r_ps = ps_r.tile([128, V2_R_CHUNK], F32, name="r_ps")
                        i_mm = 0
                        for xi in range(2):
                            for k in range(KC):
                                nc.tensor.matmul(
                                    r_ps[:],
                                    wt_sb[:, k],
                                    xr[:, :, xi * KC + k, :],
                                    start=(i_mm == 0),
                                    stop=(i_mm == 2 * KC - 1),
                                )
                                i_mm += 1
                        rls = lspool.tile([128, V2_R_CHUNK], F32, tag="ls")
                        nc.scalar.copy(rls[:], r_ps[:])

                        rw = opool.tile([128, V2_S_PER_CH, TOPK], F32, tag="rw")
                        ri = opool.tile([128, V2_S_PER_CH, TOPK], I32, tag="ri")
                        for j in range(V2_S_PER_CH):
                            epilogue_tile(rls, j, rw, ri, flag_slot=None)

                        nc.scalar.dma_start(resc_w[:, c], rw[:])
                        nc.scalar.dma_start(resc_i[:, c], ri[:])
                    if not do_rescue:
                        # AB bring-up: just make sure the gathers land
                        nc.gpsimd.wait_ge(
                            g_sem, 16 * V2_S_PER_CH * V2_N_RCH
                        )
    if True:
        # Raw Bass skips Bacc's compile passes; gpsimd extended insts
        # (dma_gather) need the GPSIMD library loads inserted and the ISA
        # bytes generated, else walrus dies with "ISA wrong length".
        import bass_rust as _bass_rust
        from concourse.library_config import all_libraries, standard

        inst_type_to_lib_mask = {}
        for lib in all_libraries:
            for inst_type in lib.instructions:
                inst_type_to_lib_mask[inst_type] = inst_type_to_lib_mask.get(
                    inst_type, 0
                ) | (1 << lib.index)
        _bass_rust.insert_library_loads(
            nc, inst_type_to_lib_mask, len(all_libraries), standard.index
        )
        # The inserted library loads are DMA-classified (ucode fetch) but
        # carry no sem update, which the race detector / BIR invariants
        # reject. Give them a dedicated sem nobody waits on.
        _lib_sems = list(nc._gate_lib_sems)
        for fn in nc.m.functions:
            for bb in fn.blocks:
                for inst in bb.instructions:
                    if type(inst).__name__ in ("InstPseudoReloadLibraryIndex",):
                        _bass_rust.then_inc(inst, _lib_sems.pop(0), 16, False)
        mybir.codegen_inst_isa_subclasses(nc)

    _split_multi_waits(nc)
    return nc


def _build_program(mode: str) -> bass.Bass:
    if mode == "v2":
        return _build_v2_program()
    if mode == "rescue":
        return _build_rescue_program()
    nc = bass.Bass()
    if mode == "split3":
        mm_dt = F16
        # hi+lo fp16 planes packed together: [g, p, plane, k, t]
        xb = nc.declare_dram_parameter("xb", [N_G, 128, 2, KC, T_G], F16,
                                       isOutput=False)
        wt = nc.declare_dram_parameter("wt", [128, KC, 2, E], F16,
                                       isOutput=False)
    else:
        mm_dt = F32R if mode == "f32r" else F32
        xb = nc.declare_dram_parameter("xb", [N_G, 128, KC, T_G], F32,
                                       isOutput=False)
        wt = nc.declare_dram_parameter("wt", [128, KC, E], F32, isOutput=False)
    w_out = nc.declare_dram_parameter("w_out", [T_CORE, TOPK], F32, isOutput=True)
    i_out = nc.declare_dram_parameter("i_out", [T_CORE, TOPK], I32, isOutput=True)

    def mmsrc(ap):
        return ap.bitcast(F32R) if mm_dt == F32R else ap

    with tile.TileContext(nc) as tc:
        with (
            tc.tile_pool(name="const", bufs=1) as const_pool,
            tc.tile_pool(name="xin", bufs=2) as xpool,
            tc.tile_pool(name="lsb", bufs=2) as lspool,
            tc.tile_pool(name="lg", bufs=4) as lgpool,
            tc.tile_pool(name="epi", bufs=4) as epool,
            tc.tile_pool(name="outg", bufs=2) as opool,
            tc.tile_pool(name="ps_l", bufs=2, space="PSUM") as ps_l,
            tc.tile_pool(name="ps_t", bufs=4, space="PSUM") as ps_t,
        ):
            ident = const_pool.tile([128, 128], F32)
            masks.make_identity(nc, ident[:])

            if mode == "split3":
                wt_sb = const_pool.tile([128, KC, 2, E], F16)
            else:
                wt_sb = const_pool.tile([128, KC, E], mm_dt)
            nc.sync.dma_start(wt_sb[:], mmsrc(wt[:]))

            for g in range(N_G):
                if mode == "split3":
                    xg = xpool.tile([128, 2, KC, T_G], F16, tag="xg")
                else:
                    xg = xpool.tile([128, KC, T_G], mm_dt, tag="xg")
                nc.sync.dma_start(xg[:], mmsrc(xb[g]))

                logitsT = ps_l.tile([E, T_G], F32, name="logitsT")
                if mode == "split3":
                    # x1@w1 + x1@w2 + x2@w1, accumulated in PSUM
                    terms = [(0, 0), (0, 1), (1, 0)]
                    n_mm = KC * len(terms)
                    i_mm = 0
                    for k in range(KC):
                        for (xi, wi) in terms:
                            nc.tensor.matmul(
                                logitsT[:],
                                wt_sb[:, k, wi, :],
                                xg[:, xi, k, :],
                                start=(i_mm == 0),
                                stop=(i_mm == n_mm - 1),
                            )
                            i_mm += 1
                else:
                    for k in range(KC):
                        nc.tensor.matmul(
                            logitsT[:],
                            wt_sb[:, k, :],
                            xg[:, k, :],
                            start=(k == 0),
                            stop=(k == KC - 1),
                        )

                ls = lspool.tile([E, T_G], F32, tag="ls")
                nc.scalar.copy(ls[:], logitsT[:])

                w_grp = opool.tile([128, T_G // 128, TOPK], F32, tag="wg")
                i_grp = opool.tile([128, T_G // 128, TOPK], I32, tag="ig")

                for j in range(T_G // 128):
                    lt_ps = ps_t.tile([128, E], F32, name="lt_ps")
                    nc.tensor.transpose(
                        lt_ps[:], ls[:, j * 128 : (j + 1) * 128], ident[:E, :E]
                    )
                    lg = lgpool.tile([128, E], F32, tag="lg")
                    nc.vector.tensor_copy(lg[:], lt_ps[:])

                    mx8 = epool.tile([128, TOPK], F32, tag="mx8")
                    nc.vector.max(mx8[:], lg[:])
                    nc.vector.max_index(
                        i_grp[:, j, :].bitcast(U32), mx8[:], lg[:]
                    )

                    negmax = epool.tile([128, 1], F32, tag="negmax")
                    nc.scalar.mul(negmax[:], mx8[:, 0:1], -1.0)

                    expall = epool.tile([128, E], F32, tag="expall")
                    denom = epool.tile([128, 1], F32, tag="denom")
                    nc.scalar.activation(
                        expall[:],
                        lg[:],
                        mybir.ActivationFunctionType.Exp,
                        bias=negmax[:],
                        accum_out=denom[:],
                    )
                    exp8 = epool.tile([128, TOPK], F32, tag="exp8")
                    nc.scalar.activation(
                        exp8[:],
                        mx8[:],
                        mybir.ActivationFunctionType.Exp,
                        bias=negmax[:],
                    )
                    r25 = epool.tile([128, 1], F32, tag="r25")
                    nc.vector.reciprocal(r25[:], denom[:])
                    nc.scalar.mul(r25[:], r25[:], ROUTE_SCALE)
                    nc.vector.tensor_scalar_mul(w_grp[:, j, :], exp8[:], r25[:])

                # outputs are tiny; keep them off the x-load DMA ring (HWDGE
                # has two physical rings — sync/SP carries the 8 MiB loads,
                # scalar/ACT carries the stores)
                nc.scalar.dma_start(
                    w_out[g * T_G : (g + 1) * T_G, :].rearrange(
                        "(j p) e -> p j e", p=128
                    ),
                    w_grp[:],
                )
                nc.scalar.dma_start(
                    i_out[g * T_G : (g + 1) * T_G, :].rearrange(
                        "(j p) e -> p j e", p=128
                    ),
                    i_grp[:],
                )

    _split_multi_waits(nc)
    return nc


_NC = {}


def _get_program(mode: str) -> bass.Bass:
    if mode not in _NC:
        _NC[mode] = _build_program(mode)
    return _NC[mode]


def _pack_x_core(shard: np.ndarray) -> np.ndarray:
    """[T_CORE, D] f32 -> [N_G, 128, KC, T_G] with
    xb[g, p, k, t] = shard[g*T_G + t, k*128 + p]."""
    v = shard.reshape(N_G, T_G, KC, 128)
    return np.ascontiguousarray(v.transpose(0, 3, 2, 1))


def _pack_w(W: np.ndarray) -> np.ndarray:
    """[E, D] -> [128, KC, E] with wt[p, k, e] = W[e, k*128 + p]."""
    v = W.reshape(E, KC, 128)
    return np.ascontiguousarray(v.transpose(2, 1, 0))


def _split16(a: np.ndarray):
    hi = a.astype(np.float16)
    lo = (a - hi.astype(np.float32)).astype(np.float16)
    return hi, lo


def _run(x: np.ndarray, W: np.ndarray, **kwargs):
    x = np.asarray(x, dtype=np.float32)
    W = np.asarray(W, dtype=np.float32)
    assert x.shape == (TOKENS, D), x.shape
    assert W.shape == (E, D), W.shape

    mode = MODE
    in_maps = []
    if mode == "v2":
        w_pack = _pack_w(W)  # [128, KC, E] f32
        w1, w2 = _split16(w_pack)
        wt_host = np.ascontiguousarray(
            np.stack([w1, w2], axis=2).reshape(128, KC, 2 * E)
        )  # [128, KC, 2E]: cols [w1 | w2]
        for c in range(N_CORES):
            shard = x[c * T_CORE : (c + 1) * T_CORE, :]
            hi = shard.astype(np.float16)
            lo = (shard - hi.astype(np.float32)).astype(np.float16)
            xb = np.ascontiguousarray(
                hi.reshape(N_G, T_G, KC, 128).transpose(0, 3, 2, 1)
            )
            xrows = np.ascontiguousarray(
                np.concatenate([hi, lo], axis=1)
            )  # [T_CORE, 2*D] f16
            in_maps.append({"xb": xb, "xrows": xrows, "wt": wt_host})
    elif mode == "rescue":
        w_pack = _pack_w(W)
        w1, w2 = _split16(w_pack)
        wt_host = np.ascontiguousarray(np.stack([w1, w2], axis=2))
        for c in range(N_CORES):
            shard = x[c * T_CORE : (c + 1) * T_CORE, :]
            hi = shard.astype(np.float16)
            lo = (shard - hi.astype(np.float32)).astype(np.float16)
            xb = np.ascontiguousarray(
                hi.reshape(N_G1, T_G1, KC, 128).transpose(0, 3, 2, 1)
            )
            xrows = np.ascontiguousarray(
                np.concatenate([hi, lo], axis=1)
            )  # [T_CORE, 2*D] f16
            in_maps.append({"xb": xb, "xrows": xrows, "wt": wt_host})
    elif mode == "split3":
        w_pack = _pack_w(W)  # [128, KC, E] f32
        w1, w2 = _split16(w_pack)
        wt_host = np.ascontiguousarray(
            np.stack([w1, w2], axis=2)
        )  # [128, KC, 2, E] f16
        for c in range(N_CORES):
            xp = _pack_x_core(x[c * T_CORE : (c + 1) * T_CORE, :])
            x1, x2 = _split16(xp)
            xb = np.ascontiguousarray(
                np.stack([x1, x2], axis=2)
            )  # [N_G, 128, 2, KC, T_G] f16
            in_maps.append({"xb": xb, "wt": wt_host})
    else:
        wt_host = _pack_w(W)
        for c in range(N_CORES):
            xb = _pack_x_core(x[c * T_CORE : (c + 1) * T_CORE, :])
            in_maps.append({"xb": xb, "wt": wt_host})

    nc = _get_program(mode)
    res = run_bass_kernel_spmd(nc, in_maps, core_ids=list(range(N_CORES)), **kwargs)

    w_parts, i_parts = [], []
    for c in range(N_CORES):
        w_c = np.asarray(res.results[c]["w_out"])
        i_c = np.asarray(res.results[c]["i_out"])
        if mode == "v2" and V2_PHASE == "full":
            # host merge of the dense rescue side buffer
            rid = np.asarray(res.results[c]["resc_id"]).reshape(128, V2_R_PP)
            rw = np.asarray(res.results[c]["resc_w"]).reshape(
                128, V2_R_PP, TOPK
            )
            ri = np.asarray(res.results[c]["resc_i"]).reshape(
                128, V2_R_PP, TOPK
            )
            valid = (rid >= 0) & (rid < T_CORE)
            ids = rid[valid]
            w_c = w_c.copy()
            i_c = i_c.copy()
            w_c[ids] = rw[valid]
            i_c[ids] = ri[valid]
        w_parts.append(w_c)
        i_parts.append(i_c)
    weights = np.concatenate(w_parts, axis=0)
    indices = np.concatenate(i_parts, axis=0)
    return weights.astype(np.float32), indices.astype(np.int32), res


def kernel(x: np.ndarray, W: np.ndarray):
    weights, indices, _ = _run(x, W)
    return weights, indices

